# revision 60
# baseline (speedup 1.0000x reference)
"""Trainium2 Bass kernel for nn_Attention_39384850104955 (single-launch).

Dense multi-head attention (B=2, S=2048, D=1024, H=16, dh=64) with a
materialized [B,H,S,S] score tensor plus additive bias, eager softmax,
and in/out projections.

Sharding: head-parallel across 8 NeuronCores; core c owns heads
{2c, 2c+1} for BOTH batches, so each bias head is read exactly once
across the fleet. Per core:
- QKV projections for its 2 heads (fp16), batch-interleaved block order
  so the first score groups can start while later blocks still project.
- scoresT = k^T q + bias. The bias is stored fp8-e4m3 (halves its HBM
  read, the dominant input) pre-tiled on host to the slab layout with
  the two heads interleaved as the two halves of an fp8 DoubleRow
  identity matmul, which injects a head's slab into PSUM at 2 elem/
  cycle/partition (half the PE cost of a bf16 identity copy).
- exp on ScalarE (bf16 out), attn@v with a fused ones-column for row
  sums (fp16), per-row normalize (DVE recip -> Pool broadcast -> DVE
  mult), attn@v/oproj interleaved across batches so PE stays fed while
  each normalize chain completes.
- Partial output projection over the core's 128 head-dims, then a
  ReduceScatter(add) across cores per (batch, seq-block) landing each
  core's final row-slice. Each RS is emitted one seq-block late so the
  in-order Pool queue never stalls a normalize broadcast on a pending
  collective. Output assembled on host from the 8 per-core row shards.
"""

import sys

sys.path.insert(0, "/opt/trn_rl_repo")

import numpy as np
import jax.numpy as jnp

import concourse.bacc as bacc
import concourse.mybir as mybir
import concourse.tile as tile
from concourse.bass_utils import run_bass_kernel_spmd

f32 = mybir.dt.float32
f16 = mybir.dt.float16
f32r = mybir.dt.float32r
f8 = mybir.dt.float8e4  # e4m3 (ml_dtypes.float8_e4m3, has inf, max 240)
bf16 = mybir.dt.float16  # fp16: same PE/DVE speed as bf16, 8x the mantissa precision

P = 128
B, S, D, H, DH = 2, 2048, 1024, 16, 64
NCORE = 8
NH2 = 2 * DH          # 128 head dims per core (2 heads)
NT = S // P           # 16 sk tiles per batch
SBLK = 512            # projection seq block
SQB = 512             # attention sq block
NQB = S // SQB        # 4 sq blocks
SCALE = 1.0 / 8.0     # 1/sqrt(dh)

Exp = mybir.ActivationFunctionType.Exp
Mult = mybir.AluOpType.mult
DoubleRow = mybir.MatmulPerfMode.DoubleRow

_CACHE = {}


def _emit_body(nc, tc, ident_s, ident16, idw8, w_s, woc_s, xT, biasr, parts,
               rss, fin, qk_pool, v_pool, rs_queue, skip=(), simsafe=False,
               expbufs=5):
    qT_s = qk_pool.tile([P, B * S], bf16, tag="qT", name="qT")  # pre-scaled
    kT_s = qk_pool.tile([P, B * S], bf16, tag="kT", name="kT")
    # double-buffered across reps: v is read until the very last attn@v of
    # a rep, so a single buffer would stall the next rep's projections
    v_s = v_pool.tile([P, 2 * NT, 2, 65], bf16, tag="v", name="v")
    # per-head ones column at free offset 64 for row sums
    nc.vector.memset(v_s[:, :, :, 64:65], 1.0)

    # ---------------- QKV projections ----------------
    with tc.tile_pool(name="xload", bufs=2) as xload, \
         tc.tile_pool(name="vstage", bufs=2) as vstage, \
         tc.tile_pool(name="pp", bufs=2, space="PSUM") as pp:
        # batch-interleaved order: scores group g needs k-tiles of BOTH
        # batches, so finish (b0-blk, b1-blk) pairs early to let the first
        # score groups start while later projections still run
        for sb in (0, 4, 1, 5, 2, 6, 3, 7):
            s0 = sb * SBLK
            xt = xload.tile([P, 8, SBLK], bf16, tag="xt", name="xt")
            if "xdma" not in skip:
                nc.sync.dma_start(
                    xt[:], xT[:, s0:s0 + SBLK].rearrange("(c p) n -> p c n", p=P))
            else:
                nc.vector.memset(xt[0:1, 0, 0:1], 0.0)
            pq = pp.tile([P, SBLK], f32, tag="pq", name="pq")
            pk = pp.tile([P, SBLK], f32, tag="pk", name="pk")
            pv = pp.tile([P, SBLK], f32, tag="pv", name="pv")
            nch = 1 if "proj" in skip else 8
            for c in range(nch):
                st, sp = (c == 0), (c == nch - 1)
                nc.tensor.matmul(pq[:], w_s[:, 0, c, :], xt[:, c, :],
                                 start=st, stop=sp)
                nc.tensor.matmul(pk[:], w_s[:, 1, c, :], xt[:, c, :],
                                 start=st, stop=sp)
                nc.tensor.matmul(pv[:], w_s[:, 2, c, :], xt[:, c, :],
                                 start=st, stop=sp)
            if "evac" in skip:
                nc.vector.tensor_scalar_mul(qT_s[0:1, s0:s0 + 1], pq[0:1, 0:1],
                                            SCALE)
                nc.vector.tensor_copy(kT_s[0:1, s0:s0 + 1], pk[0:1, 0:1])
                continue
            nc.vector.tensor_scalar_mul(qT_s[:, s0:s0 + SBLK], pq[:], SCALE)
            nc.vector.tensor_copy(kT_s[:, s0:s0 + SBLK], pk[:])
            # v: transpose [dout, s] -> [s, dout] tiles (512-wide streams
            # keep the PE's stationary loads amortized), store bf16 + ones
            vst = vstage.tile([P, SBLK], bf16, tag="vst", name="vst")
            nc.vector.tensor_copy(vst[:], pv[:])
            for a in range(4):
                pvt = pp.tile([P, P], bf16, tag="pvt", name="pvt")
                nc.tensor.matmul(pvt[:],
                                 vst[:, a * P:(a + 1) * P], ident16[:],
                                 is_transpose=True, start=True, stop=True)
                g = sb * 4 + a  # global sk tile 0..31 (= bb*16 + t)
                nc.vector.tensor_copy(v_s[:, g, 0, 0:64], pvt[:, 0:64])
                nc.vector.tensor_copy(v_s[:, g, 1, 0:64], pvt[:, 64:128])

    # ---------------- attention + fused output projection ----------------
    with tc.tile_pool(name="bias", bufs=3) as bias_pool, \
         tc.tile_pool(name="expp", bufs=expbufs) as exp_pool, \
         tc.tile_pool(name="nrm", bufs=2) as nrm_pool, \
         tc.tile_pool(name="ost", bufs=3) as ost_pool, \
         tc.tile_pool(name="ptp", bufs=3) as pt_pool, \
         tc.tile_pool(name="sc", bufs=2, space="PSUM") as sc_pool, \
         tc.tile_pool(name="av", bufs=2, space="PSUM") as av_pool, \
         tc.tile_pool(name="pf", bufs=2, space="PSUM") as pf_pool:
        hsl = [slice(0, 64), slice(64, 128)]
        for sqb in range(NQB):
            sq0 = sqb * SQB
            # both heads' bias tiles interleaved as the two DoubleRow halves;
            # host pre-tiled to the slab layout so the load is contiguous
            slb = bias_pool.tile([P, NT, 2, SQB], f8, tag="slab", name="slab")
            if "bdma" not in skip:
                nc.sync.dma_start(slb[:], biasr[sqb])
            else:
                nc.vector.memset(slb[0:1, 0, 0, 0:1], 0.0)
            expt = {}
            for bb in range(2):
                for h in range(2):
                    expt[bb, h] = exp_pool.tile([P, NT * SQB], bf16,
                                                tag="exp",
                                                name=f"exp_{bb}_{h}")
            ocs = {bb: ost_pool.tile([P, SQB], bf16, tag="oc", name=f"oc{bb}")
                   for bb in range(2)}
            # batch-0 attn@v accumulates INSIDE the g-loop (one group behind
            # the exps): the g-loop is ACT-paced, so these matmuls ride in
            # PE slack instead of serializing after the loop
            ilv = "attnv" not in skip
            pa0 = {h: av_pool.tile([65, SQB], f32, tag="av", name="av")
                   for h in range(2)} if ilv else None

            def av_partial(g):
                for j in range(2):
                    t = g * 2 + j
                    for h in range(2):
                        nc.tensor.matmul(
                            pa0[h][:], v_s[:, t, h, :],
                            expt[0, h][:, t * SQB:(t + 1) * SQB],
                            start=(t == 0), stop=(t == NT - 1))

            for g in range(8):
                psg = [sc_pool.tile([P, 2 * SQB], f32, tag="sc",
                                    name=f"sc{h}") for h in range(2)]
                for h in range(2):
                    for j in range(2):
                        t = g * 2 + j
                        dst = psg[h][:, j * SQB:(j + 1) * SQB]
                        if "inject" in skip:
                            nc.tensor.matmul(dst[:, 0:64], idw8[:, :, h, :],
                                             slb[:, t, :, 0:P],
                                             start=True, stop=False,
                                             perf_mode=DoubleRow)
                        else:
                            nc.tensor.matmul(dst, idw8[:, :, h, :],
                                             slb[:, t, :, :],
                                             start=True, stop=False,
                                             perf_mode=DoubleRow)
                # batch-0 scores: A/B adjacent for row-group concurrency
                for j in range(2):
                    t = g * 2 + j
                    for h in range(2):
                        nc.tensor.matmul(
                            psg[h][:, j * SQB:(j + 1) * SQB],
                            kT_s[hsl[h], t * P:(t + 1) * P],
                            qT_s[hsl[h], sq0:sq0 + SQB],
                            start=False, stop=simsafe)
                for h in range(2):
                    if "exp" in skip:
                        nc.scalar.activation(
                            expt[0, h][:, 2 * g * SQB:2 * g * SQB + P],
                            psg[h][:, 0:P], Exp)
                    else:
                        nc.scalar.activation(
                            expt[0, h][:, 2 * g * SQB:(2 * g + 2) * SQB],
                            psg[h][:], Exp)
                if "b1mm" not in skip:
                    for h in range(2):
                        for j in range(2):
                            t = g * 2 + j
                            nc.tensor.matmul(
                                psg[h][:, j * SQB:(j + 1) * SQB],
                                idw8[:, :, h, :], slb[:, t, :, :],
                                start=True, stop=False,
                                perf_mode=DoubleRow)
                    for j in range(2):
                        t = g * 2 + j
                        for h in range(2):
                            nc.tensor.matmul(
                                psg[h][:, j * SQB:(j + 1) * SQB],
                                kT_s[hsl[h], S + t * P:S + (t + 1) * P],
                                qT_s[hsl[h], S + sq0:S + sq0 + SQB],
                                start=False, stop=(True if simsafe
                                                   else (j == 1)))
                for h in range(2):
                    if "exp" in skip:
                        nc.scalar.activation(
                            expt[1, h][:, 2 * g * SQB:2 * g * SQB + P],
                            psg[h][:, 0:P], Exp)
                    else:
                        nc.scalar.activation(
                            expt[1, h][:, 2 * g * SQB:(2 * g + 2) * SQB],
                            psg[h][:], Exp)
                if ilv and g > 0:
                    av_partial(g - 1)
            if ilv:
                av_partial(7)

            def norm0(h):
                recip = nrm_pool.tile([1, SQB], f32, tag="recip",
                                      name="recip")
                nc.vector.reciprocal(recip[:], pa0[h][64:65, :])
                rbc = nrm_pool.tile([64, SQB], f32, tag="rbc", name="rbc")
                nc.gpsimd.partition_broadcast(rbc[:], recip[:])
                nc.vector.tensor_tensor(ocs[0][hsl[h], :], pa0[h][0:64, :],
                                        rbc[:], Mult)

            def attnv(bb, h):
                pa = av_pool.tile([65, SQB], f32, tag="av", name="av")
                nts = 1 if "attnv" in skip else NT
                for t in range(nts):
                    nc.tensor.matmul(
                        pa[:], v_s[:, bb * NT + t, h, :],
                        expt[bb, h][:, t * SQB:(t + 1) * SQB],
                        start=(t == 0), stop=(t == nts - 1))
                recip = nrm_pool.tile([1, SQB], f32, tag="recip",
                                      name="recip")
                nc.vector.reciprocal(recip[:], pa[64:65, :])
                rbc = nrm_pool.tile([64, SQB], f32, tag="rbc", name="rbc")
                nc.gpsimd.partition_broadcast(rbc[:], recip[:])
                nc.vector.tensor_tensor(ocs[bb][hsl[h], :], pa[0:64, :],
                                        rbc[:], Mult)

            def oproj(bb):
                # partial output projection for rows (bb, sq0:sq0+SQB)
                nrt = 1 if "oproj" in skip else 4
                for rt in range(nrt):
                    for nh in range(2):
                        pfin = pf_pool.tile([P, 512], f32, tag="pf",
                                            name="pf")
                        nc.tensor.matmul(pfin[:],
                                         ocs[bb][:, rt * P:(rt + 1) * P],
                                         woc_s[:, nh * 512:(nh + 1) * 512],
                                         start=True, stop=True)
                        pt = pt_pool.tile([P, 512], f16, tag="pt", name="pt")
                        nc.vector.tensor_copy(pt[:], pfin[:])
                        nc.sync.dma_start(
                            parts[sqb][bb * SQB + rt * P:
                                       bb * SQB + (rt + 1) * P,
                                       nh * 512:(nh + 1) * 512], pt[:])

            # order keeps PE fed while each oproj's normalize chain
            # (DVE recip -> Pool broadcast -> DVE mult) completes
            if ilv:
                norm0(0)
                norm0(1)
                attnv(1, 0)
                attnv(1, 1)
                oproj(0)
                oproj(1)
            else:
                attnv(0, 0)
                attnv(0, 1)
                attnv(1, 0)
                oproj(0)
                attnv(1, 1)
                oproj(1)
            if "rs" not in skip:
                def emit_rs(sqb=sqb):
                    nc.gpsimd.collective_compute(
                        "ReduceScatter", mybir.AluOpType.add,
                        replica_groups=[list(range(NCORE))],
                        ins=[parts[sqb]], outs=[rss[sqb]])
                    nc.sync.dma_start(fin[sqb], rss[sqb])
                # delay the RS by one sqb: Pool runs broadcasts(s_k) before
                # RS(s_{k-1}), whose parts-DMA inputs are then long complete,
                # so the in-order Pool queue never stalls the normalize chain
                if rs_queue:
                    rs_queue.pop(0)()
                rs_queue.append(emit_rs)
            else:
                nc.sync.dma_start(fin[sqb],
                                  parts[sqb][0:B * SQB // NCORE, :])


def build_full(repeat=1, skip=(), simsafe=False, vbufs=2, qkbufs=1,
               expbufs=6):
    nc = bacc.Bacc("TRN2", target_bir_lowering=False, debug=False,
                   num_devices=NCORE)
    xT = nc.dram_tensor("xT", [D, B * S], bf16, kind="ExternalInput").ap()
    wT = nc.dram_tensor("wT", [3, D, P], bf16, kind="ExternalInput").ap()
    biasr = nc.dram_tensor("biasr", [NQB, P, NT, 2, SQB], f8,
                           kind="ExternalInput").ap()
    identr = nc.dram_tensor("identr", [P, P], f32r, kind="ExternalInput").ap()
    identw8r = nc.dram_tensor("identw8", [P, 2, 2, P], f8,
                              kind="ExternalInput").ap()
    woc = nc.dram_tensor("woc", [P, D], bf16, kind="ExternalInput").ap()
    fin = nc.dram_tensor("fin", [NQB, B * SQB // NCORE, D], f16,
                         kind="ExternalOutput").ap()
    parts = [nc.dram_tensor(f"part{q}", [B * SQB, D], f16).ap()
             for q in range(NQB)]
    rss = [nc.dram_tensor(f"rs{q}", [B * SQB // NCORE, D], f16).ap()
           for q in range(NQB)]

    with tile.TileContext(nc) as tc:
        with tc.tile_pool(name="const", bufs=1) as const_pool, \
             tc.tile_pool(name="qk", bufs=qkbufs) as qk_pool, \
             tc.tile_pool(name="vp", bufs=vbufs) as v_pool:
            ident_s = const_pool.tile([P, P], f32r, tag="ident", name="ident")
            nc.sync.dma_start(ident_s[:], identr)
            ident16 = const_pool.tile([P, P], bf16, tag="ident16",
                                      name="ident16")
            nc.vector.tensor_copy(ident16[:], ident_s[:].bitcast(f32))
            idw8 = const_pool.tile([P, 2, 2, P], f8, tag="idw8", name="idw8")
            nc.sync.dma_start(idw8[:], identw8r)
            w_s = const_pool.tile([P, 3, 8, P], bf16, tag="w", name="w")
            nc.sync.dma_start(w_s[:], wT.rearrange("w (c p) m -> p w c m", p=P))
            woc_s = const_pool.tile([P, D], bf16, tag="woc", name="woc")
            nc.sync.dma_start(woc_s[:], woc)
            rs_queue = []
            for _rep in range(repeat):
                _emit_body(nc, tc, ident_s, ident16, idw8, w_s, woc_s, xT,
                           biasr, parts, rss, fin, qk_pool, v_pool, rs_queue,
                           skip=skip, simsafe=simsafe, expbufs=expbufs)
            for emit_rs in rs_queue:
                emit_rs()

    nc.compile()
    return nc


def _get(name, builder):
    if name not in _CACHE:
        _CACHE[name] = builder()
    return _CACHE[name]


def make_in_maps(hidden_states, bias, Wq, Wk, Wv, Wo):
    import ml_dtypes
    f8np = ml_dtypes.float8_e4m3

    xT = np.asarray(jnp.asarray(hidden_states.reshape(B * S, D).T,
                                dtype=jnp.float16))
    # bias in fp8 e4m3, pre-tiled to the slab layout [sqb, p, t, h, n]
    # (sk = t*128+p, sq = sqb*512+n); adjacent head pairs stay adjacent in
    # the h dim — they are the two DoubleRow halves of the inject matmul
    bias8 = (bias[0].transpose(2, 0, 1)          # [sk, H, sq]
             .reshape(NT, P, H, NQB, SQB)
             .transpose(3, 1, 0, 2, 4)           # [sqb, p, t, H, n]
             .astype(f8np))
    ident = np.eye(P, dtype=np.float32)
    idw8 = np.zeros((P, 2, 2, P), dtype=f8np)
    ar = np.arange(P)
    idw8[ar, 0, 0, ar] = 1.0
    idw8[ar, 1, 1, ar] = 1.0
    in_maps = []
    for c in range(NCORE):
        r0 = c * NH2
        wTc = np.stack([np.asarray(jnp.asarray(W[r0:r0 + NH2, :].T,
                                               dtype=jnp.float16))
                        for W in (Wq, Wk, Wv)])
        in_maps.append({
            "xT": xT,
            "wT": wTc,
            "biasr": np.ascontiguousarray(bias8[:, :, :, 2 * c:2 * c + 2]),
            "identr": ident,
            "identw8": idw8,
            "woc": np.asarray(jnp.asarray(Wo[:, r0:r0 + NH2].T,
                                          dtype=jnp.float16)),
        })
    return in_maps


def assemble(results):
    RW = B * SQB // NCORE  # 128 rows per core per sqb-chunk
    out = np.empty((B * S, D), dtype=np.float32)
    for c in range(NCORE):
        finc = np.asarray(results[c]["fin"], dtype=np.float32)
        bb, ci = c // 4, c % 4
        for sqb in range(NQB):
            r0 = bb * S + sqb * SQB + ci * RW
            out[r0:r0 + RW] = finc[sqb]
    return out.reshape(B, S, D)


def kernel(hidden_states, bias, Wq, Wk, Wv, Wo):
    hidden_states = np.ascontiguousarray(hidden_states, dtype=np.float32)
    bias = np.ascontiguousarray(bias, dtype=np.float32)
    Wq = np.ascontiguousarray(Wq, dtype=np.float32)
    Wk = np.ascontiguousarray(Wk, dtype=np.float32)
    Wv = np.ascontiguousarray(Wv, dtype=np.float32)
    Wo = np.ascontiguousarray(Wo, dtype=np.float32)

    nc = _get("full", build_full)
    in_maps = make_in_maps(hidden_states, bias, Wq, Wk, Wv, Wo)
    res = run_bass_kernel_spmd(nc, in_maps, list(range(NCORE))).results
    return assemble(res)

